# revision 8
# baseline (speedup 1.0000x reference)
"""Trainium2 Bass kernel for nn_ModalityAdaptiveModule (fp8-DoubleRow).

Reference computation (B=2, S=4096, D=512):
    tn = LN(text, g_t, b_t); im = LN(img, g_i, b_i)
    norms = concat([tn, im]); K/V/Q projections; attn = softmax(q@K.T/sqrt(D))
    x = attn@V; x = x@Wo.T + bo; out = concat([LN(x)*g_t+b_t, LN(x)*g_i+b_i])

Sharding: 8 cores = (attention batch b in 0..3) x (query half h in 0..1),
identical to the baseline kernel.

Numerics/speed design (cost model: fp8e4+DoubleRow matmul = 0.5 cyc/row over
2 contraction tiles = 4x bf16):
  - x, A, projections, Wo: bf16 (error ~0.4% per element)
  - K, Q stored as fp8 hi+lo pairs; scores via 3 DoubleRow insts per 2
    d-tiles (hi@hi paired + 2 cross terms; lo@lo dropped) -> 1.33x bf16,
    error ~1.5e-3
  - U = exp(scores - 3.5): hi part Uh = fp8(exp(.)) direct from ACT; a
    second exp emits U in bf16 and Ul = U - Uh is one fused STT op (DVE
    2x_2p). c=3.5 keeps max ~e^5=148 < 240 across all batches; per-query
    normalization cancels e^-c exactly. sums use Uh only — the resulting
    per-query scale error is killed by the final LN's row normalization.
  - V stored hi+lo fp8; attnV = 3 DoubleRow insts per k-tile pair
    (Vh@Uh + Vl@Uh + Vh@Ul; lo@lo dropped) -> 1.33x bf16
  - NO explicit softmax normalization: py = xT@Wo^T (+ sums*bo' when
    bo'!=0 via a rank-1 ones-DR sums matmul) is per-row proportional to the
    true pre-LN activation, and the final LN is scale-invariant per row
  - final LN affine skipped on device when gamma==1, beta==0 (host check;
    general path emitted otherwise)
"""

import numpy as np
import ml_dtypes

import concourse.bass as bass
import concourse.mybir as mybir
import concourse.tile as tile
from concourse import bacc
from concourse.bass_utils import run_bass_kernel_spmd

AF = mybir.ActivationFunctionType
OP = mybir.AluOpType
DR = mybir.MatmulPerfMode.DoubleRow

# Pin ALL activations to the exp/ln table set (only Exp + Ln are used) so no
# LoadActFuncSet is ever inserted mid-kernel.
import concourse.hw_specs as _hw_specs
import functools as _functools

_ORIG_GET_ACT_TABLES = _hw_specs.get_activation_tables


@_functools.cache
def _pinned_act_tables(module_arch):
    full = _ORIG_GET_ACT_TABLES(module_arch)
    keep = "natural_log_exp_and_others"
    return {name: (funcs if name == keep else set())
            for name, funcs in full.items()}


_hw_specs.get_activation_tables = _pinned_act_tables
bacc.get_activation_tables = _pinned_act_tables

F32 = mybir.dt.float32
BF16 = mybir.dt.bfloat16
E4 = mybir.dt.float8e4

D = 512
S = 4096          # keys per batch
TQ = 2048         # queries per core
DT = 4            # d tiles of 128
NKT = S // 128    # 32 key tiles
NPAIR = NKT // 2  # 16 key-tile pairs
TC = 512          # phase-1 token chunk
NCH = S // TC     # 8 chunks
NSUB = TC // 128  # 4 subtiles per chunk
EPS = 1e-5
CEXP = 3.5        # exp offset: U = exp(s - CEXP)


def build_kernel(fast_affine=True, skip_bop=True):
    nc = bacc.Bacc("TRN2", target_bir_lowering=False, debug=False,
                   enable_asserts=True, num_devices=8)

    x_d = nc.dram_tensor("x", [S, D], BF16, kind="ExternalInput").ap()
    gqt_d = nc.dram_tensor("gqt", [D, D], BF16, kind="ExternalInput").ap()
    gkt_d = nc.dram_tensor("gkt", [D, D], BF16, kind="ExternalInput").ap()
    gvt_d = nc.dram_tensor("gvt", [D, D], BF16, kind="ExternalInput").ap()
    wot_d = nc.dram_tensor("wot", [D, D], BF16, kind="ExternalInput").ap()
    cq_d = nc.dram_tensor("cq16", [D], F32, kind="ExternalInput").ap()
    ck_d = nc.dram_tensor("ck", [D], F32, kind="ExternalInput").ap()
    bop_d = nc.dram_tensor("bop", [1, D], BF16, kind="ExternalInput").ap()
    identf_d = nc.dram_tensor("identf", [128, 128], F32, kind="ExternalInput").ap()
    identb_d = nc.dram_tensor("identb", [128, 128], BF16, kind="ExternalInput").ap()
    ones8_d = nc.dram_tensor("ones8", [128, 2, 128], E4, kind="ExternalInput").ap()
    g2t_d = nc.dram_tensor("g2t", [D], F32, kind="ExternalInput").ap()
    b2t_d = nc.dram_tensor("b2t", [D], F32, kind="ExternalInput").ap()
    g2i_d = nc.dram_tensor("g2i", [D], F32, kind="ExternalInput").ap()
    b2i_d = nc.dram_tensor("b2i", [D], F32, kind="ExternalInput").ap()
    out_d = nc.dram_tensor("out2", [2, TQ, D], F32, kind="ExternalOutput").ap()

    def bcast(vec_ap, parts=128):
        return bass.AP(tensor=vec_ap.tensor, offset=vec_ap.offset,
                       ap=[[0, parts]] + list(vec_ap.ap))

    with tile.TileContext(nc) as tc:
        with (
            tc.tile_pool(name="persist", bufs=1) as persist,
            tc.tile_pool(name="resident", bufs=1) as resident,
        ):
            # ---- critical path first: first x chunk, then identity ----
            xc0 = persist.tile([128, 4, D], BF16)
            nc.sync.dma_start(
                xc0[:, 0, :], x_d[0:128, :].rearrange("(s p) d -> p (s d)", p=128))
            nc.sync.dma_start(
                xc0[:, 1:4, :], x_d[128:TC, :].rearrange("(s p) d -> p s d", p=128))
            identb = persist.tile([128, 128], BF16)
            nc.sync.dma_start(identb[:], identb_d)
            if not skip_bop:
                ones8 = persist.tile([128, 2, 128], E4)
                nc.sync.dma_start(ones8[:], ones8_d)
            eps_t = persist.tile([128, 1], F32)
            nc.vector.memset(eps_t[:], EPS)
            # dummy activation: loads the exp/ln table set while DMAs stream
            warm = persist.tile([128, 1], F32)
            nc.scalar.activation(warm[:], eps_t[:], AF.Exp, scale=1.0)
            cexp_t = persist.tile([128, 1], F32)
            nc.vector.memset(cexp_t[:], -CEXP)
            cq_s = persist.tile([128, DT], F32)
            nc.sync.dma_start(cq_s[:], cq_d.rearrange("(o p) -> p o", p=128))
            ck_s = persist.tile([128, DT], F32)
            nc.sync.dma_start(ck_s[:], ck_d.rearrange("(o p) -> p o", p=128))
            if not skip_bop:
                bop_sb = persist.tile([1, D], BF16)
                nc.sync.dma_start(bop_sb[:], bop_d)

            # ---- resident big tensors ----
            KTh = resident.tile([128, DT, S], E4)
            KTl = resident.tile([128, DT, S], E4)
            QTh = resident.tile([128, DT, TQ], E4)
            QTl = resident.tile([128, DT, TQ], E4)
            V8h = resident.tile([128, NPAIR, 2, D], E4)
            V8l = resident.tile([128, NPAIR, 2, D], E4)
            wot_s = resident.tile([128, DT, D], BF16)

            def rsig_lnexp(pool, var_ap, tag):
                """1/sqrt(var+eps) via exp(-0.5*ln(var+eps)) on ACT."""
                lnv = pool.tile([128, 1], F32, tag=f"lnv{tag}", name=f"lnv{tag}")
                nc.scalar.activation(lnv[:], var_ap, AF.Ln, bias=eps_t[:, 0:1],
                                     scale=1.0)
                rs = pool.tile([128, 1], F32, tag=f"rsx{tag}", name=f"rsx{tag}")
                nc.scalar.activation(rs[:], lnv[:], AF.Exp, scale=-0.5)
                return rs

            # ================= PHASE 1: LN + transpose + QKV =================
            with (
                tc.tile_pool(name="p1w", bufs=1) as p1w,
                tc.tile_pool(name="p1x", bufs=2) as p1x,
                tc.tile_pool(name="p1s", bufs=4) as p1s,
                tc.tile_pool(name="p1f", bufs=8) as p1f,
                tc.tile_pool(name="p1ps", bufs=1, space="PSUM") as p1ps,
                tc.tile_pool(name="p1pk", bufs=4, space="PSUM") as p1pk,
                tc.tile_pool(name="p1pv", bufs=3, space="PSUM") as p1pv,
            ):
                xc_tiles = {}

                def fetch_x(c):
                    if c == 0:
                        xc_tiles[0] = xc0
                        return
                    xc = p1x.tile([128, NSUB, D], BF16, tag="xc",
                                  name=f"xc{c}", bufs=5)
                    nc.sync.dma_start(
                        xc[:], x_d[c * TC:(c + 1) * TC, :].rearrange(
                            "(s p) d -> p s d", p=128))
                    xc_tiles[c] = xc

                def ln_transpose(c):
                    xc = xc_tiles.pop(c)
                    Ac = p1x.tile([128, NSUB, D], BF16, tag="ac", name=f"ac{c}",
                                  bufs=4)
                    AcT = p1x.tile([128, DT, TC], BF16, tag="act", name=f"act{c}",
                                   bufs=4)
                    for s in range(NSUB):
                        stats = p1s.tile([128, 6], F32, tag="st", name=f"st{c}_{s}")
                        nc.vector.bn_stats(stats[:], xc[:, s, :])
                        mv = p1s.tile([128, 2], F32, tag="mv", name=f"mv{c}_{s}")
                        nc.vector.bn_aggr(mv[:], stats[:])
                        rs = rsig_lnexp(p1s, mv[:, 1:2], "1")
                        nmr = p1s.tile([128, 1], F32, tag="nmr", name=f"nmr{c}_{s}")
                        nc.vector.tensor_scalar(
                            out=nmr[:], in0=mv[:, 0:1], scalar1=rs[:, 0:1],
                            scalar2=-1.0, op0=OP.mult, op1=OP.mult)
                        nc.vector.tensor_scalar(
                            out=Ac[:, s, :], in0=xc[:, s, :], scalar1=rs[:, 0:1],
                            scalar2=nmr[:, 0:1], op0=OP.mult, op1=OP.add)
                        tp = p1ps.tile([128, DT, 128], BF16, tag="tp",
                                       name=f"tp{c}_{s}")
                        for dt in range(DT):
                            nc.tensor.transpose(
                                tp[:, dt, :], Ac[:, s, dt * 128:(dt + 1) * 128],
                                identb[:])
                        nc.scalar.copy(AcT[:, :, s * 128:(s + 1) * 128], tp[:])
                    return AcT

                def projections(c, AcT):
                    # K: transposed out [d_out part, tokens]; hi/lo fp8 evac.
                    # hi on ACT, lo residual (STT) on DVE: GPSIMD cannot
                    # read PSUM on hardware.
                    for o in range(DT):
                        pk = p1pk.tile([128, TC], F32, tag="pk", name=f"pk{c}_{o}")
                        for i in range(DT):
                            nc.tensor.matmul(
                                pk[:], gkt_s[:, i, o * 128:(o + 1) * 128],
                                AcT[:, i, :], start=(i == 0), stop=(i == DT - 1))
                        kf = p1f.tile([128, TC], F32, tag="kf",
                                      name=f"kf{c}_{o}")
                        nc.scalar.activation(kf[:], pk[:], AF.Identity,
                                             bias=ck_s[:, o:o + 1], scale=1.0)
                        kh = KTh[:, o, c * TC:(c + 1) * TC]
                        nc.gpsimd.tensor_copy(kh, kf[:])
                        nc.vector.scalar_tensor_tensor(
                            out=KTl[:, o, c * TC:(c + 1) * TC], in0=kf[:],
                            scalar=1.0, in1=kh, op0=OP.mult, op1=OP.subtract)
                    # V: natural out [tokens part, d]; hi/lo fp8 evac (no bias)
                    for s in range(NSUB):
                        pv = p1pv.tile([128, D], F32, tag="pv", name=f"pv{c}_{s}")
                        for i in range(DT):
                            nc.tensor.matmul(
                                pv[:], AcT[:, i, s * 128:(s + 1) * 128],
                                gvt_s[:, i, :], start=(i == 0), stop=(i == DT - 1))
                        vf = p1f.tile([128, D], F32, tag="vf",
                                      name=f"vf{c}_{s}")
                        nc.scalar.copy(vf[:], pv[:])
                        vh = V8h[:, 2 * c + s // 2, s % 2, :]
                        nc.gpsimd.tensor_copy(vh, vf[:])
                        nc.vector.scalar_tensor_tensor(
                            out=V8l[:, 2 * c + s // 2, s % 2, :], in0=vf[:],
                            scalar=1.0, in1=vh, op0=OP.mult, op1=OP.subtract)
                    # Q (first half of chunks only = this core's queries)
                    if c < NCH // 2:
                        for o in range(DT):
                            pq = p1pk.tile([128, TC], F32, tag="pk",
                                           name=f"pq{c}_{o}")
                            for i in range(DT):
                                nc.tensor.matmul(
                                    pq[:], gqt_s[:, i, o * 128:(o + 1) * 128],
                                    AcT[:, i, :], start=(i == 0), stop=(i == DT - 1))
                            qf = p1f.tile([128, TC], F32, tag="kf",
                                           name=f"qf{c}_{o}")
                            nc.scalar.activation(qf[:], pq[:], AF.Identity,
                                                 bias=cq_s[:, o:o + 1], scale=1.0)
                            qh = QTh[:, o, c * TC:(c + 1) * TC]
                            nc.gpsimd.tensor_copy(qh, qf[:])
                            nc.vector.scalar_tensor_tensor(
                                out=QTl[:, o, c * TC:(c + 1) * TC], in0=qf[:],
                                scalar=1.0, in1=qh, op0=OP.mult, op1=OP.subtract)

                # skewed: transpose chunk c while projecting chunk c-1.
                # q-chunks (Q evac = 8 extra Pool ops) interleave with
                # non-q chunks so per-step engine load stays balanced.
                order = [c for pair in zip(range(NCH // 2), range(NCH // 2, NCH))
                         for c in pair]
                fetch_x(order[0])
                fetch_x(order[1])
                fetch_x(order[2])
                fetch_x(order[3])
                gkt_s = p1w.tile([128, DT, D], BF16)
                nc.sync.dma_start(gkt_s[:], gkt_d.rearrange("(i p) o -> p i o", p=128))
                gvt_s = p1w.tile([128, DT, D], BF16)
                nc.sync.dma_start(gvt_s[:], gvt_d.rearrange("(i p) o -> p i o", p=128))
                gqt_s = p1w.tile([128, DT, D], BF16)
                nc.sync.dma_start(gqt_s[:], gqt_d.rearrange("(i p) o -> p i o", p=128))
                prev, prev_c = ln_transpose(order[0]), order[0]
                for i, c in enumerate(order[1:]):
                    if i + 4 < NCH:
                        fetch_x(order[i + 4])
                    cur = ln_transpose(c)
                    projections(prev_c, prev)
                    prev, prev_c = cur, c
                projections(prev_c, prev)

            nc.sync.dma_start(wot_s[:], wot_d.rearrange("(i p) o -> p i o", p=128))

            # ============ PHASE 2/3: attention + out-proj + final LN ============
            import contextlib
            with (
                contextlib.ExitStack() as _p2stack,
                tc.tile_pool(name="p2u", bufs=7) as p2u,
                tc.tile_pool(name="p2ub", bufs=5) as p2ub,
                tc.tile_pool(name="p2ul", bufs=7) as p2ul,
                tc.tile_pool(name="p2s", bufs=3) as p2s,
                tc.tile_pool(name="p2y", bufs=2) as p2y,
                tc.tile_pool(name="p2o", bufs=2) as p2o,
                tc.tile_pool(name="p2st", bufs=3) as p2st,
                tc.tile_pool(name="p2sum", bufs=2) as p2sum,
                tc.tile_pool(name="p2c", bufs=1) as p2c,
                tc.tile_pool(name="psc", bufs=3, space="PSUM") as psc,
                tc.tile_pool(name="pxv", bufs=1, space="PSUM") as pxv,
                tc.tile_pool(name="psum_y", bufs=(1 if skip_bop else 1),
                             space="PSUM") as psum_y,
            ):
                psm_p = (None if skip_bop else
                         _p2stack.enter_context(
                             tc.tile_pool(name="psm_p", bufs=1, space="PSUM")))
                if not fast_affine:
                    g2t_rep = p2c.tile([128, D], F32)
                    nc.gpsimd.dma_start(g2t_rep[:], bcast(g2t_d))
                    b2t_rep = p2c.tile([128, D], F32)
                    nc.gpsimd.dma_start(b2t_rep[:], bcast(b2t_d))
                    g2i_rep = p2c.tile([128, D], F32)
                    nc.gpsimd.dma_start(g2i_rep[:], bcast(g2i_d))
                    b2i_rep = p2c.tile([128, D], F32)
                    nc.gpsimd.dma_start(b2i_rep[:], bcast(b2i_d))

                def oproj_ln(q0, w, xT, sums_sb, last=False):
                    # py = xT@Wo^T (+ sums*bo' when bo'!=0) is, per row, an
                    # exact positive multiple (1/sums) of the true pre-LN2
                    # activation — and LN is scale-invariant per row, so no
                    # softmax normalization (reciprocal/diag) is needed at all.
                    for j in range(w // 128):
                        py = psum_y.tile([128, D], F32, tag="py",
                                         name=f"py{q0}_{j}")
                        for dt in range(DT):
                            nc.tensor.matmul(
                                py[:], xT[:, dt, j * 128:(j + 1) * 128],
                                wot_s[:, dt, :], start=(dt == 0),
                                stop=(skip_bop and dt == DT - 1))
                        if not skip_bop:
                            # rank-1 bo' row: sums*bo' (scales with the row)
                            nc.tensor.matmul(
                                py[:], sums_sb[0:1, j * 128:(j + 1) * 128],
                                bop_sb[:], start=False, stop=True)
                        stats = p2st.tile([128, 6], F32, tag="st2",
                                          name=f"st2_{q0}_{j}")
                        nc.vector.bn_stats(stats[:], py[:])
                        mv = p2st.tile([128, 2], F32, tag="mv2",
                                       name=f"mv2_{q0}_{j}")
                        nc.vector.bn_aggr(mv[:], stats[:])
                        rs2 = rsig_lnexp(p2st, mv[:, 1:2], "2")
                        nmr2 = p2st.tile([128, 1], F32, tag="nmr2",
                                         name=f"nmr2_{q0}_{j}")
                        nc.vector.tensor_scalar(
                            out=nmr2[:], in0=mv[:, 0:1], scalar1=rs2[:, 0:1],
                            scalar2=-1.0, op0=OP.mult, op1=OP.mult)
                        n2 = p2y.tile([128, D], F32, tag="n2", name=f"n2_{q0}_{j}")
                        nc.scalar.activation(n2[:], py[:], AF.Identity,
                                             bias=nmr2[:, 0:1],
                                             scale=rs2[:, 0:1])
                        r0 = q0 + j * 128
                        if fast_affine:
                            nc.sync.dma_start(out_d[0, r0:r0 + 128, :], n2[:])
                            # final block: second copy on the DVE DGE queue so
                            # the two tail DMAs overlap
                            eng = nc.scalar if last else nc.sync
                            eng.dma_start(out_d[1, r0:r0 + 128, :], n2[:])
                        else:
                            for m, (g_rep, b_rep) in enumerate(
                                    [(g2t_rep, b2t_rep), (g2i_rep, b2i_rep)]):
                                om = p2o.tile([128, D], F32, tag=f"om{m}",
                                              name=f"om{m}_{q0}_{j}")
                                nc.vector.tensor_mul(om[:], n2[:], g_rep[:])
                                nc.vector.tensor_add(om[:], om[:], b_rep[:])
                                nc.sync.dma_start(out_d[m, r0:r0 + 128, :], om[:])

                prev_oproj = None
                BLOCKS = [(0, 512), (512, 512), (1024, 512),
                          (1536, 256), (1792, 128), (1920, 128)]
                for blk, (q0, w) in enumerate(BLOCKS):
                    pxs = [pxv.tile([128, w], F32, tag=f"px{dt}",
                                    name=f"px{dt}_{blk}") for dt in range(DT)]
                    psm = (None if skip_bop else
                           psm_p.tile([128, w], F32, tag="psm", name=f"psm{blk}"))
                    Us = [None] * NPAIR
                    Uls = [None] * NPAIR
                    for k in range(NKT + 9):
                        if k == 15 and prev_oproj is not None:
                            oproj_ln(*prev_oproj)
                            prev_oproj = None
                        if k < NKT:
                            p = k // 2
                            if k % 2 == 0:
                                Us[p] = p2u.tile([128, 2, w], E4, tag="ut",
                                                 name=f"ut{blk}_{p}")
                                Uls[p] = p2ul.tile([128, 2, w], E4, tag="ul",
                                                   name=f"ul{blk}_{p}")
                            ps = psc.tile([128, w], F32, tag="ps",
                                          name=f"ps{blk}_{k}")
                            ks = slice(k * 128, (k + 1) * 128)
                            # 3-inst corrected fp8 scores per 2 d-tiles
                            for j in range(2):
                                dj = slice(2 * j, 2 * j + 2)
                                nc.tensor.matmul(
                                    ps[:], KTh[:, dj, ks], QTh[:, dj, q0:q0 + w],
                                    start=(j == 0), stop=False, perf_mode=DR)
                                nc.tensor.matmul(
                                    ps[:], KTl[:, dj, ks], QTh[:, dj, q0:q0 + w],
                                    start=False, stop=False, perf_mode=DR)
                                nc.tensor.matmul(
                                    ps[:], KTh[:, dj, ks], QTl[:, dj, q0:q0 + w],
                                    start=False, stop=(j == 1), perf_mode=DR)
                            ubf = p2ub.tile([128, w], BF16, tag="ub",
                                            name=f"ub{blk}_{k}")
                            nc.scalar.activation(ubf[:], ps[:], AF.Exp,
                                                 bias=cexp_t[:, 0:1],
                                                 scale=1.0 / 16.0)
                            nc.vector.tensor_copy(Us[p][:, k % 2, :], ubf[:])
                            nc.vector.scalar_tensor_tensor(
                                out=Uls[p][:, k % 2, :], in0=ubf[:],
                                scalar=1.0, in1=Us[p][:, k % 2, :],
                                op0=OP.mult, op1=OP.subtract)
                        if k >= 9 and k % 2 == 1:
                            # attnV for pair p, skewed one extra k-slot so the
                            # exp2 -> Ul chain never blocks the in-order PE
                            # queue; Ul-dependent insts issued last
                            p = (k - 9) // 2
                            U = Us[p]
                            Ul = Uls[p]
                            for dt in range(DT):
                                ds = slice(dt * 128, (dt + 1) * 128)
                                nc.tensor.matmul(
                                    pxs[dt][:], V8h[:, p, :, ds], U[:],
                                    start=(p == 0), stop=False, perf_mode=DR)
                                nc.tensor.matmul(
                                    pxs[dt][:], V8l[:, p, :, ds], U[:],
                                    start=False, stop=False, perf_mode=DR)
                            if not skip_bop:
                                # sums (for the bo' row), replicated across
                                # partitions of psm
                                nc.tensor.matmul(
                                    psm[:], ones8[:], U[:], start=(p == 0),
                                    stop=(p == NPAIR - 1), perf_mode=DR)
                            for dt in range(DT):
                                ds = slice(dt * 128, (dt + 1) * 128)
                                nc.tensor.matmul(
                                    pxs[dt][:], V8h[:, p, :, ds], Ul[:],
                                    start=False, stop=(p == NPAIR - 1),
                                    perf_mode=DR)
                            Us[p] = None
                            Uls[p] = None
                    # evacuate x_T (bf16), sums (bf16), recip(sums)
                    xT = p2s.tile([128, DT, w], BF16, tag="xt", name=f"xt{blk}")
                    for dt in range(DT):
                        if dt < 2:
                            nc.scalar.copy(xT[:, dt, :], pxs[dt][:])
                        else:
                            nc.vector.tensor_copy(xT[:, dt, :], pxs[dt][:])
                    if skip_bop:
                        sums_sb = None
                    else:
                        sums_sb = p2sum.tile([1, w], BF16, tag="ssb",
                                             name=f"ssb{blk}")
                        nc.vector.tensor_copy(sums_sb[:], psm[0:1, :])
                    prev_oproj = (q0, w, xT, sums_sb)
                oproj_ln(*prev_oproj, last=True)
    nc.compile()
    return nc


_NC_CACHE = {}


def _get_nc(fast_affine=True, skip_bop=True):
    key = (fast_affine, skip_bop)
    if key not in _NC_CACHE:
        _NC_CACHE[key] = build_kernel(fast_affine, skip_bop)
    return _NC_CACHE[key]


def _prep_core_inputs(text, img, ln_t_g, ln_t_b, ln_i_g, ln_i_b,
                      Wq, bq, Wkt, bkt, Wvt, bvt, Wki, bki, Wvi, bvi, Wo, bo):
    s = np.float32(D) ** -0.5
    bf = ml_dtypes.bfloat16
    e4 = ml_dtypes.float8_e4m3
    identf = np.eye(128, dtype=np.float32)
    in_maps = []
    for core in range(8):
        b, h = core // 2, core % 2
        m_t = b < 2
        x = np.asarray(text[b] if m_t else img[b - 2], np.float32)
        if h == 1:
            x = np.concatenate([x[TQ:], x[:TQ]], axis=0)
        g = np.asarray(ln_t_g if m_t else ln_i_g, np.float32)
        bb = np.asarray(ln_t_b if m_t else ln_i_b, np.float32)
        Wk, bk = (Wkt, bkt) if m_t else (Wki, bki)
        Wv, bv = (Wvt, bvt) if m_t else (Wvi, bvi)
        Wq_, bq_, Wk, bk, Wv, bv, Wo_, bo_ = [
            np.asarray(a, np.float32) for a in (Wq, bq, Wk, bk, Wv, bv, Wo, bo)]
        cv = Wv @ bb + bv
        in_maps.append({
            "x": np.ascontiguousarray(x.astype(bf)),
            "gqt": np.ascontiguousarray(((Wq_ * g[None, :]).T * (16.0 * s))
                                        .astype(bf)),
            "gkt": np.ascontiguousarray((Wk * g[None, :]).T.astype(bf)),
            "gvt": np.ascontiguousarray((Wv * g[None, :]).T.astype(bf)),
            "wot": np.ascontiguousarray(Wo_.T.astype(bf)),
            "cq16": np.ascontiguousarray(16.0 * ((Wq_ @ bb + bq_) * s)),
            "ck": np.ascontiguousarray(Wk @ bb + bk),
            "bop": np.ascontiguousarray((cv @ Wo_.T + bo_)[None, :].astype(bf)),
            "identf": identf,
            "identb": identf.astype(bf),
            "ones8": np.ones((128, 2, 128), e4),
            "g2t": np.ascontiguousarray(np.asarray(ln_t_g, np.float32)),
            "b2t": np.ascontiguousarray(np.asarray(ln_t_b, np.float32)),
            "g2i": np.ascontiguousarray(np.asarray(ln_i_g, np.float32)),
            "b2i": np.ascontiguousarray(np.asarray(ln_i_b, np.float32)),
        })
    return in_maps


def kernel(**inputs):
    kr = kernel_raw(**inputs)
    return kr[0]


def kernel_raw(**inputs):
    """Returns (full_output, BassKernelResults)."""
    import time as _time
    fast = (np.allclose(np.asarray(inputs["ln_t_g"]), 1.0)
            and np.allclose(np.asarray(inputs["ln_t_b"]), 0.0)
            and np.allclose(np.asarray(inputs["ln_i_g"]), 1.0)
            and np.allclose(np.asarray(inputs["ln_i_b"]), 0.0))
    in_maps = _prep_core_inputs(**inputs)
    skip_bop = all(np.allclose(m["bop"].astype(np.float32), 0.0)
                   for m in in_maps)
    nc = _get_nc(fast, skip_bop)
    if fast:
        # fast-affine kernel writes LN(y) for both modalities; the affine
        # (gamma=1, beta=0) is the identity so outputs coincide
        pass
    res = None
    last_exc = None
    for attempt in range(6):
        try:
            res = run_bass_kernel_spmd(nc, in_maps, core_ids=list(range(8)))
            break
        except Exception as e:  # transient device wedge self-heals in ~1-3 min
            last_exc = e
            if "UNAVAILABLE" not in str(e) and "INTERNAL" not in str(e):
                raise
            _time.sleep(30)
    if res is None:
        raise last_exc
    out = np.zeros((8, S, D), np.float32)
    for core in range(8):
        b, h = core // 2, core % 2
        o2 = res.results[core]["out2"]
        out[b, h * TQ:(h + 1) * TQ] = o2[0]
        out[4 + b, h * TQ:(h + 1) * TQ] = o2[1]
    return out, res
